# revision 13
# baseline (speedup 1.0000x reference)
"""Trainium2 Bass kernel for the CrossLayer problem.

Math: reference computes, per row x (length D), with cur_0 = x:
    cur_{i+1} = sum(cur_i) * (w_i ⊙ x) + b_i + x        (i = 0..L-1)
Only the scalar s_i = sum(cur_i) couples elements, so with
    X   = sum(x)                  (per row)
    W_i = x · w_i                 (per row, i = 0..L-2)
    c_i = sum(b_i)
the recursion collapses to scalars:
    S_0 = X;  S_{i+1} = S_i * W_i + c_i + X
and the output is a single elementwise pass:
    out = x ⊙ (S_{L-1} * w_{L-1} + 1) + b_{L-1}
        = (x ⊙ w_{L-1}) * S_{L-1} + x + b_{L-1}

Kernel layout (per core, pure data parallel over batch, 8 pairs of
(128, 1024) row tiles):
  - The whole 8 MiB input and 8 MiB output stay resident in SBUF (no
    buffer reuse), so every load doorbell issues at t=0 with no semaphore
    wait and the SDMA engines drain the full input back-to-back; store
    doorbells never wait on buffer frees, only on their producer.
  - PE transposes each 128x128 chunk of both tiles of a pair into 4 PSUM
    banks; PSUM->SBUF xT copies ([128,1024] each, fp32 -> f32r cast) go
    one on ACT and one on DVE. Dots [X, W0, W1, W2] via 8 accumulating
    f32r matmuls with N=256 moving; small PE transposes put them
    row-major.
  - The scalar recursion S3 runs as 6 tiny tensor_scalar ops on DVE
    (~0.1 us each) instead of 1.9 us of serial GPSIMD ops. Output h0
    takes the t-path (ACT t=S3*w3+1 via per-partition scale, DVE t⊙x);
    output h1 takes the fused path (GPSIMD precomputes xw3=x⊙w3 right
    after the load — off the critical path — then one
    scalar_tensor_tensor pass out=(xw3*S3)+x).
  - dots/dT PSUM->SBUF copies ride ACT so DVE's big k1 copy never stalls
    the PE's dT transpose.
  - ALL DMA doorbells ride the sync (SP) queue. Interleaved row pairing
    (partition r of tile h holds DRAM row base + 2r + h) lets one
    dma_start move a whole 1MB pair each way.
"""

import os
import numpy as np

B, D, L = 16384, 1024, 4
N_CORES = 8
RPC = B // N_CORES          # rows per core
P = 128                     # partitions
N_TILES = RPC // P          # 16
N_PAIRS = N_TILES // 2      # 8
N_CHUNKS = D // P           # 8

_built = {}


def _build_nc(b_zero: bool):
    import concourse.bass as bass
    import concourse.bacc as bacc
    import concourse.mybir as mybir
    from concourse import tile

    f32 = mybir.dt.float32
    f32r = mybir.dt.float32r
    Alu = mybir.AluOpType
    Act = mybir.ActivationFunctionType

    # Bacc (not raw Bass): its compile() legalizes semaphore waits — TRN2
    # matmuls encode at most one sync wait (walrus S3_LW struct).
    nc = bacc.Bacc(
        "TRN2", target_bir_lowering=False, debug=False, num_devices=N_CORES
    )
    x_d = nc.dram_tensor("x", [RPC, D], f32, kind="ExternalInput")
    wpk_d = nc.dram_tensor("wpk", [P, N_CHUNKS * 4], f32r, kind="ExternalInput")
    w3bc_d = nc.dram_tensor("w3bc", [P, D], f32, kind="ExternalInput")
    ident_d = nc.dram_tensor("ident", [P, P], f32, kind="ExternalInput")
    if not b_zero:
        cvec_d = nc.dram_tensor("cvec", [P, 4], f32, kind="ExternalInput")
        b3bc_d = nc.dram_tensor("b3bc", [P, D], f32, kind="ExternalInput")
    out_d = nc.dram_tensor("out", [RPC, D], f32, kind="ExternalOutput")

    # Stage lags (iterations behind the pair's transposes).
    DOTS_LAG, REC_LAG, T_LAG, OUT_LAG, DMA_LAG = 1, 2, 3, 4, 5

    with tile.TileContext(nc) as tc:
        with (
            tc.tile_pool(name="consts", bufs=1) as consts,
            tc.tile_pool(name="xin", bufs=N_PAIRS) as xin_pool,
            tc.tile_pool(name="xw3", bufs=OUT_LAG + 2) as xw3_pool,
            tc.tile_pool(name="tp", bufs=3) as t_pool,
            tc.tile_pool(name="xts", bufs=2) as xts_pool,
            tc.tile_pool(name="outp", bufs=N_PAIRS if b_zero else 4) as out_pool,
            tc.tile_pool(name="small", bufs=5) as small_pool,
            tc.tile_pool(name="ps_t", bufs=2, space=bass.MemorySpace.PSUM) as ps_t,
            tc.tile_pool(name="ps_d", bufs=3, space=bass.MemorySpace.PSUM) as ps_d,
            tc.tile_pool(name="ps_s", bufs=1, space=bass.MemorySpace.PSUM) as ps_s,
        ):
            pre_x = {}

            def load_pair(p, split):
                # Interleaved row pairing: partition r of tile h holds DRAM
                # row p*256 + 2r + h, so one dma_start covers the whole pair
                # (the [P, 2, D] SBUF pattern matches DRAM row-major 1:1).
                xp = xin_pool.tile([P, 2, D], f32, name="xp")
                r0 = p * 2 * P
                if split:
                    for q in range(2):
                        nc.sync.dma_start(
                            xp[:, :, q * (D // 2):(q + 1) * (D // 2)],
                            x_d[r0:r0 + 2 * P, q * (D // 2):(q + 1) * (D // 2)],
                        )
                else:
                    nc.sync.dma_start(xp[:], x_d[r0:r0 + 2 * P, :])
                pre_x[p] = xp

            # consts first (ident gates the very first transpose)
            ident = consts.tile([P, P], f32)
            nc.sync.dma_start(ident[:], ident_d[:])
            wpk = consts.tile([P, N_CHUNKS * 4], f32r)
            nc.sync.dma_start(wpk[:], wpk_d[:])

            load_pair(0, split=True)

            w3bc = consts.tile([P, D], f32)
            nc.sync.dma_start(w3bc[:], w3bc_d[:])
            if not b_zero:
                cvec = consts.tile([P, 4], f32)
                nc.sync.dma_start(cvec[:], cvec_d[:])
                b3bc = consts.tile([P, D], f32)
                nc.sync.dma_start(b3bc[:], b3bc_d[:])

            # ALL remaining pair loads issue back-to-back right here (~0.65 us
            # per doorbell on SP, vs ~2.7 us SDMA drain per 1 MiB pair, so
            # issue stays ahead of the engines). None has a semaphore wait.
            for p in range(1, N_PAIRS):
                load_pair(p, split=False)

            # Prologue: absorb each const-DMA completion into one engine
            # observation up front, so steady-state instructions never need
            # two fresh semaphore waits (walrus: one sync wait per matmul).
            prol0 = ps_t.tile([P, 1024], f32, name="prol0", tag="xt_ps")
            nc.tensor.transpose(prol0[0:P, 0:P], ident[:], ident[:])
            prol1 = ps_d.tile([4, 2 * P], f32, name="prol1", tag="dots_ps")
            nc.tensor.matmul(
                prol1[:, 0:32], wpk[:, 0:4], wpk[:], start=True, stop=True
            )
            prolc = small_pool.tile([P, 1], f32, name="prolc")
            nc.scalar.activation(prolc[:], w3bc[:, 0:1], Act.Copy)
            prolv = small_pool.tile([P, 1], f32, name="prolv")
            nc.vector.tensor_mul(prolv[:], w3bc[:, 0:1], w3bc[:, 0:1])
            prolg = small_pool.tile([P, 1], f32, name="prolg")
            nc.gpsimd.tensor_copy(prolg[:], w3bc[:, 0:1])
            if not b_zero:
                prolg2 = small_pool.tile([P, 1], f32, name="prolg2")
                nc.gpsimd.tensor_copy(prolg2[:], cvec[:, 0:1])
                prolb = small_pool.tile([P, 1], f32, name="prolb")
                nc.vector.tensor_mul(prolb[:], b3bc[:, 0:1], b3bc[:, 0:1])

            # Per-pair state carried between pipeline stages
            st = {}

            def emit_xw3(p):
                """GPSIMD: xw3 = x_h1 ⊙ w3. Depends only on the pair's load
                — ready work that fills GPSIMD's queue while the PE
                transposes."""
                xp = pre_x[p]
                xw = xw3_pool.tile([P, D], f32, name="xw")
                nc.gpsimd.tensor_mul(xw[:], xp[:, 1, :], w3bc[:])
                st[p] = {"xw": xw}

            def emit_transposes(p):
                """PE: 16 chunk transposes into 2 two-bank PSUM tiles; one
                [128,1024] copy to SBUF (f32r) each on ACT and DVE."""
                xp = pre_x[p]
                xts = xts_pool.tile([P, 4 * 512], f32r, name="xts")
                for k in range(2):
                    xt_ps = ps_t.tile([P, 1024], f32, name="xt_ps", tag="xt_ps")
                    for cc in range(4):
                        c = 4 * k + cc
                        for h in range(2):
                            nc.tensor.transpose(
                                xt_ps[:, cc * 256 + h * P:cc * 256 + (h + 1) * P],
                                xp[:, h, c * P:(c + 1) * P],
                                ident[:],
                            )
                    dst = xts[:, k * 1024:(k + 1) * 1024]
                    if k == 0:
                        nc.scalar.copy(dst, xt_ps[:])
                    else:
                        nc.vector.tensor_copy(dst, xt_ps[:])
                st[p]["xts"] = xts

            def emit_dots(p):
                """PE: 8 accumulating f32r matmuls (N=256) + 2 small
                transposes; ACT does the small PSUM->SBUF copies so DVE's
                big k1 copy never delays the PE's dT transpose."""
                xts = st[p]["xts"]
                dots_ps = ps_d.tile([4, 2 * P], f32, name="dots_ps", tag="dots_ps")
                for c in range(N_CHUNKS):
                    nc.tensor.matmul(
                        dots_ps[:],
                        wpk[:, c * 4:(c + 1) * 4],
                        xts[:, c * 256:(c + 1) * 256],
                        start=(c == 0),
                        stop=(c == N_CHUNKS - 1),
                    )
                dots = small_pool.tile([4, 2 * P], f32, name="dots")
                nc.scalar.copy(dots[:], dots_ps[:])
                dT_ps = ps_s.tile([P, 8], f32, name="dT_ps")
                for h in range(2):
                    nc.tensor.transpose(
                        dT_ps[:, h * 4:(h + 1) * 4],
                        dots[:, h * P:(h + 1) * P],
                        ident[0:4, 0:4],
                    )
                dT = small_pool.tile([P, 8], f32, name="dT")
                nc.scalar.copy(dT[:], dT_ps[:])
                st[p]["dT"] = dT
                del st[p]["xts"]

            def emit_rec(p):
                """DVE: the whole scalar recursion as ONE tensor_tensor_scan
                per half (state = W_t * state + X along the free dim)."""
                dT = st[p].pop("dT")
                svec = small_pool.tile([P, 8], f32, name="svec")
                for h in range(2):
                    X = dT[:, 4 * h:4 * h + 1]
                    if b_zero:
                        data1 = X.broadcast_to([P, 3])
                    else:
                        avec = small_pool.tile([P, 8], f32, name="avec")
                        nc.vector.tensor_add(
                            avec[:, 4 * h:4 * h + 3],
                            X.broadcast_to([P, 3]),
                            cvec[:, 0:3],
                        )
                        data1 = avec[:, 4 * h:4 * h + 3]
                    nc.vector.tensor_tensor_scan(
                        svec[:, 4 * h:4 * h + 3],
                        dT[:, 4 * h + 1:4 * h + 4],
                        data1,
                        X,
                        Alu.mult,
                        Alu.add,
                    )
                st[p]["svec"] = svec

            def emit_t(p):
                """ACT: t_h0 = S3*w3 + 1 (activation Copy with per-partition
                scale, immediate bias). Emitted first in the iteration so
                ACT does this ready work while the PE transposes."""
                svec = st[p]["svec"]
                tp = t_pool.tile([P, D], f32, name="tp")
                nc.scalar.activation(
                    tp[:], w3bc[:], Act.Copy, bias=1.0, scale=svec[:, 2:3]
                )
                st[p]["tp"] = tp

            # GPSIMD elementwise is ~2.05x slower per element than DVE, so
            # GPSIMD gets xw3 + the small slice of mul_h0; DVE the rest.
            CUT = 560

            def emit_out(p):
                """Output: h1 fused on DVE (out = (xw3*S3) + x via
                scalar_tensor_tensor, a DVE-only opcode on V3); h0 = t ⊙ x
                split DVE/GPSIMD."""
                svec = st[p].pop("svec")
                tp = st[p].pop("tp")
                xw = st[p].pop("xw")
                xp = pre_x[p]
                out_sb = out_pool.tile([P, 2, D], f32, name="out_sb")
                nc.vector.scalar_tensor_tensor(
                    out_sb[:, 1, :], xw[:], svec[:, 6:7], xp[:, 1, :],
                    Alu.mult, Alu.add,
                )
                nc.vector.tensor_mul(
                    out_sb[:, 0, 0:CUT], tp[:, 0:CUT], xp[:, 0, 0:CUT]
                )
                nc.gpsimd.tensor_mul(
                    out_sb[:, 0, CUT:D], tp[:, CUT:D], xp[:, 0, CUT:D]
                )
                if not b_zero:
                    out2 = out_pool.tile([P, 2, D], f32, name="out2")
                    for h in range(2):
                        nc.vector.tensor_add(
                            out2[:, h, :], out_sb[:, h, :], b3bc[:]
                        )
                    out_sb = out2
                st[p]["out"] = out_sb

            def emit_outdma(p):
                out_sb = st.pop(p)["out"]
                r0 = p * 2 * P
                nc.sync.dma_start(out_d[r0:r0 + 2 * P, :], out_sb[:])

            # Software-pipelined emission. Stage lags are chosen so every
            # instruction's producers finished >= 1 iteration earlier.
            # Dep-old stages (out) are emitted BEFORE the iteration's
            # transposes/copies so each engine's in-order stream fills its
            # wait-for-PE window with ready work instead of idling in it.
            def _stage(f, p):
                if 0 <= p < N_PAIRS:
                    f(p)

            for p in range(N_PAIRS + DMA_LAG + 1):
                _stage(emit_xw3, p)
                _stage(emit_t, p - T_LAG)
                _stage(emit_rec, p - REC_LAG)
                _stage(emit_out, p - OUT_LAG)
                _stage(emit_transposes, p)
                _stage(emit_dots, p - DOTS_LAG)
                _stage(emit_outdma, p - DMA_LAG)
    nc.compile()
    return nc


def _get_nc(b_zero: bool):
    if b_zero not in _built:
        _built[b_zero] = _build_nc(b_zero)
    return _built[b_zero]


def _host_prep(w, b, b_zero):
    # Wpk[p, c*4+i] packs column i of [ones, w0, w1, w2] for D-chunk c
    M = np.empty((D, 4), dtype=np.float32)
    M[:, 0] = 1.0
    M[:, 1] = w[0]
    M[:, 2] = w[1]
    M[:, 3] = w[2]
    wpk = np.ascontiguousarray(
        M.reshape(N_CHUNKS, P, 4).transpose(1, 0, 2).reshape(P, N_CHUNKS * 4)
    )
    w3bc = np.ascontiguousarray(np.broadcast_to(w[3], (P, D)).astype(np.float32))
    ident = np.eye(P, dtype=np.float32)
    extras = {}
    if not b_zero:
        c = b.sum(axis=1).astype(np.float32)  # (L,)
        extras["cvec"] = np.ascontiguousarray(np.broadcast_to(c, (P, L)))
        extras["b3bc"] = np.ascontiguousarray(
            np.broadcast_to(b[3], (P, D)).astype(np.float32)
        )
    return wpk, w3bc, ident, extras


def kernel(inputs, w, b):
    from concourse.bass_utils import run_bass_kernel_spmd

    x = np.ascontiguousarray(np.asarray(inputs, dtype=np.float32).reshape(B, D))
    w = np.asarray(w, dtype=np.float32)
    b = np.asarray(b, dtype=np.float32)
    b_zero = not b.any()

    nc = _get_nc(b_zero)
    wpk, w3bc, ident, extras = _host_prep(w, b, b_zero)

    in_maps = []
    for i in range(N_CORES):
        m = {
            "x": x[i * RPC:(i + 1) * RPC],
            "wpk": wpk,
            "w3bc": w3bc,
            "ident": ident,
        }
        m.update(extras)
        in_maps.append(m)

    trace = bool(int(os.environ.get("KERNEL_TRACE", "0")))
    kwargs = {}
    if trace:
        kwargs = {"trace": True, "trace_cores": [0]}
    res = run_bass_kernel_spmd(nc, in_maps, core_ids=list(range(N_CORES)), **kwargs)
    if trace:
        kernel.last_results = res
    return np.concatenate([r["out"] for r in res.results], axis=0)


# revision 16
# speedup vs baseline: 1.0090x; 1.0090x over previous
"""Trainium2 Bass kernel for the CrossLayer problem.

Math: reference computes, per row x (length D), with cur_0 = x:
    cur_{i+1} = sum(cur_i) * (w_i ⊙ x) + b_i + x        (i = 0..L-1)
Only the scalar s_i = sum(cur_i) couples elements, so with
    X   = sum(x)                  (per row)
    W_i = x · w_i                 (per row, i = 0..L-2)
    c_i = sum(b_i)
the recursion collapses to scalars:
    S_0 = X;  S_{i+1} = S_i * W_i + c_i + X
and the output is a single elementwise pass:
    out = x ⊙ (S_{L-1} * w_{L-1} + 1) + b_{L-1}
        = (x ⊙ w_{L-1}) * S_{L-1} + x + b_{L-1}

Kernel layout (per core, pure data parallel over batch, 8 pairs of
(128, 1024) row tiles):
  - The whole 8 MiB input and 8 MiB output stay resident in SBUF (no
    buffer reuse), so every load doorbell issues at t=0 with no semaphore
    wait and the SDMA engines drain the full input back-to-back; store
    doorbells never wait on buffer frees, only on their producer.
  - PE transposes each 128x128 chunk of both tiles of a pair into 4 PSUM
    banks; PSUM->SBUF xT copies ([128,1024] each, fp32 -> f32r cast) go
    one on ACT and one on DVE. Dots [X, W0, W1, W2] via 8 accumulating
    f32r matmuls with N=256 moving; small PE transposes put them
    row-major.
  - The scalar recursion S3 runs as ONE tensor_tensor_scan per half on
    DVE. Both PSUM->SBUF xT copies ride ACT (PSUM reads are 1x on every
    engine). Both t = S3*w3 + 1 halves run on DVE as tensor_scalar,
    which hits the fp32 2x_2P perf mode (SBUF-only, single tensor input,
    both read ports) - 0.53 us/half instead of 1.0+. The muls t ⊙ x are
    1x tensor_tensor ops, split ~50/50 by columns between DVE and
    GPSIMD (GPSIMD is ~2x slower per element).
  - ALL DMA doorbells ride the sync (SP) queue. Interleaved row pairing
    (partition r of tile h holds DRAM row base + 2r + h) lets one
    dma_start move a whole 1MB pair each way.
"""

import os
import numpy as np

B, D, L = 16384, 1024, 4
N_CORES = 8
RPC = B // N_CORES          # rows per core
P = 128                     # partitions
N_TILES = RPC // P          # 16
N_PAIRS = N_TILES // 2      # 8
N_CHUNKS = D // P           # 8

_built = {}


def _build_nc(b_zero: bool):
    import concourse.bass as bass
    import concourse.bacc as bacc
    import concourse.mybir as mybir
    from concourse import tile

    f32 = mybir.dt.float32
    f32r = mybir.dt.float32r
    Alu = mybir.AluOpType
    Act = mybir.ActivationFunctionType

    # Bacc (not raw Bass): its compile() legalizes semaphore waits — TRN2
    # matmuls encode at most one sync wait (walrus S3_LW struct).
    nc = bacc.Bacc(
        "TRN2", target_bir_lowering=False, debug=False, num_devices=N_CORES
    )
    x_d = nc.dram_tensor("x", [RPC, D], f32, kind="ExternalInput")
    wpk_d = nc.dram_tensor("wpk", [P, N_CHUNKS * 4], f32r, kind="ExternalInput")
    w3bc_d = nc.dram_tensor("w3bc", [P, D], f32, kind="ExternalInput")
    ident_d = nc.dram_tensor("ident", [P, P], f32, kind="ExternalInput")
    if not b_zero:
        cvec_d = nc.dram_tensor("cvec", [P, 4], f32, kind="ExternalInput")
        b3bc_d = nc.dram_tensor("b3bc", [P, D], f32, kind="ExternalInput")
    out_d = nc.dram_tensor("out", [RPC, D], f32, kind="ExternalOutput")

    # Stage lags (iterations behind the pair's transposes).
    DOTS_LAG, REC_LAG, T_LAG, OUT_LAG, DMA_LAG = 1, 2, 3, 4, 5

    with tile.TileContext(nc) as tc:
        with (
            tc.tile_pool(name="consts", bufs=1) as consts,
            tc.tile_pool(name="xin", bufs=N_PAIRS) as xin_pool,
            tc.tile_pool(name="tp", bufs=3) as t_pool,
            tc.tile_pool(name="xts", bufs=2) as xts_pool,
            tc.tile_pool(name="outp", bufs=N_PAIRS if b_zero else 4) as out_pool,
            tc.tile_pool(name="small", bufs=5) as small_pool,
            tc.tile_pool(name="ps_t", bufs=2, space=bass.MemorySpace.PSUM) as ps_t,
            tc.tile_pool(name="ps_d", bufs=3, space=bass.MemorySpace.PSUM) as ps_d,
            tc.tile_pool(name="ps_s", bufs=1, space=bass.MemorySpace.PSUM) as ps_s,
        ):
            pre_x = {}

            def load_pair(p, split):
                # Interleaved row pairing: partition r of tile h holds DRAM
                # row p*256 + 2r + h, so one dma_start covers the whole pair
                # (the [P, 2, D] SBUF pattern matches DRAM row-major 1:1).
                xp = xin_pool.tile([P, 2, D], f32, name="xp")
                r0 = p * 2 * P
                if split:
                    for q in range(2):
                        nc.sync.dma_start(
                            xp[:, :, q * (D // 2):(q + 1) * (D // 2)],
                            x_d[r0:r0 + 2 * P, q * (D // 2):(q + 1) * (D // 2)],
                        )
                else:
                    nc.sync.dma_start(xp[:], x_d[r0:r0 + 2 * P, :])
                pre_x[p] = xp

            # consts first (ident gates the very first transpose)
            ident = consts.tile([P, P], f32)
            nc.sync.dma_start(ident[:], ident_d[:])
            wpk = consts.tile([P, N_CHUNKS * 4], f32r)
            nc.sync.dma_start(wpk[:], wpk_d[:])

            load_pair(0, split=True)

            w3bc = consts.tile([P, D], f32)
            nc.sync.dma_start(w3bc[:], w3bc_d[:])
            if not b_zero:
                cvec = consts.tile([P, 4], f32)
                nc.sync.dma_start(cvec[:], cvec_d[:])
                b3bc = consts.tile([P, D], f32)
                nc.sync.dma_start(b3bc[:], b3bc_d[:])

            # ALL remaining pair loads issue back-to-back right here (~0.65 us
            # per doorbell on SP, vs ~2.7 us SDMA drain per 1 MiB pair, so
            # issue stays ahead of the engines). None has a semaphore wait.
            for p in range(1, N_PAIRS):
                load_pair(p, split=False)

            # Prologue: absorb each const-DMA completion into one engine
            # observation up front, so steady-state instructions never need
            # two fresh semaphore waits (walrus: one sync wait per matmul).
            prol0 = ps_t.tile([P, 1024], f32, name="prol0", tag="xt_ps")
            nc.tensor.transpose(prol0[0:P, 0:P], ident[:], ident[:])
            prol1 = ps_d.tile([4, 2 * P], f32, name="prol1", tag="dots_ps")
            nc.tensor.matmul(
                prol1[:, 0:32], wpk[:, 0:4], wpk[:], start=True, stop=True
            )
            prolc = small_pool.tile([P, 1], f32, name="prolc")
            nc.scalar.activation(prolc[:], w3bc[:, 0:1], Act.Copy)
            prolv = small_pool.tile([P, 1], f32, name="prolv")
            nc.vector.tensor_mul(prolv[:], w3bc[:, 0:1], w3bc[:, 0:1])
            prolg = small_pool.tile([P, 1], f32, name="prolg")
            nc.gpsimd.tensor_copy(prolg[:], w3bc[:, 0:1])
            if not b_zero:
                prolg2 = small_pool.tile([P, 1], f32, name="prolg2")
                nc.gpsimd.tensor_copy(prolg2[:], cvec[:, 0:1])
                prolb = small_pool.tile([P, 1], f32, name="prolb")
                nc.vector.tensor_mul(prolb[:], b3bc[:, 0:1], b3bc[:, 0:1])

            # Per-pair state carried between pipeline stages
            st = {}

            def emit_transposes(p):
                """PE: 16 chunk transposes into 2 two-bank PSUM tiles; both
                [128,1024] PSUM->SBUF copies (f32r cast) on ACT."""
                xp = pre_x[p]
                st[p] = {}
                xts = xts_pool.tile([P, 4 * 512], f32r, name="xts")
                for k in range(2):
                    xt_ps = ps_t.tile([P, 1024], f32, name="xt_ps", tag="xt_ps")
                    for cc in range(4):
                        c = 4 * k + cc
                        for h in range(2):
                            nc.tensor.transpose(
                                xt_ps[:, cc * 256 + h * P:cc * 256 + (h + 1) * P],
                                xp[:, h, c * P:(c + 1) * P],
                                ident[:],
                            )
                    nc.scalar.copy(xts[:, k * 1024:(k + 1) * 1024], xt_ps[:])
                st[p]["xts"] = xts

            def emit_dots(p):
                """PE: 8 accumulating f32r matmuls (N=256) + 2 small
                transposes; ACT does the small PSUM->SBUF copies so DVE's
                big k1 copy never delays the PE's dT transpose."""
                xts = st[p]["xts"]
                dots_ps = ps_d.tile([4, 2 * P], f32, name="dots_ps", tag="dots_ps")
                for c in range(N_CHUNKS):
                    nc.tensor.matmul(
                        dots_ps[:],
                        wpk[:, c * 4:(c + 1) * 4],
                        xts[:, c * 256:(c + 1) * 256],
                        start=(c == 0),
                        stop=(c == N_CHUNKS - 1),
                    )
                dots = small_pool.tile([4, 2 * P], f32, name="dots")
                nc.scalar.copy(dots[:], dots_ps[:])
                dT_ps = ps_s.tile([P, 8], f32, name="dT_ps")
                for h in range(2):
                    nc.tensor.transpose(
                        dT_ps[:, h * 4:(h + 1) * 4],
                        dots[:, h * P:(h + 1) * P],
                        ident[0:4, 0:4],
                    )
                dT = small_pool.tile([P, 8], f32, name="dT")
                nc.vector.tensor_copy(dT[:], dT_ps[:])
                st[p]["dT"] = dT
                del st[p]["xts"]

            def emit_rec(p):
                """DVE: the whole scalar recursion as ONE tensor_tensor_scan
                per half (state = W_t * state + X along the free dim)."""
                dT = st[p].pop("dT")
                svec = small_pool.tile([P, 8], f32, name="svec")
                for h in range(2):
                    X = dT[:, 4 * h:4 * h + 1]
                    if b_zero:
                        data1 = X.broadcast_to([P, 3])
                    else:
                        avec = small_pool.tile([P, 8], f32, name="avec")
                        nc.vector.tensor_add(
                            avec[:, 4 * h:4 * h + 3],
                            X.broadcast_to([P, 3]),
                            cvec[:, 0:3],
                        )
                        data1 = avec[:, 4 * h:4 * h + 3]
                    nc.vector.tensor_tensor_scan(
                        svec[:, 4 * h:4 * h + 3],
                        dT[:, 4 * h + 1:4 * h + 4],
                        data1,
                        X,
                        Alu.mult,
                        Alu.add,
                    )
                st[p]["svec"] = svec

            def emit_t(p):
                """DVE: t_h = S3_h*w3 + 1 per half via tensor_scalar —
                single-tensor-input fp32 op from SBUF hits the 2x_2P DVE
                perf mode (~0.53 us/half)."""
                svec = st[p]["svec"]
                tp = t_pool.tile([P, 2, D], f32, name="tp")
                for h in range(2):
                    nc.vector.tensor_scalar(
                        tp[:, h, :], w3bc[:], svec[:, 4 * h + 2:4 * h + 3],
                        1.0, Alu.mult, Alu.add,
                    )
                st[p]["tp"] = tp

            # GPSIMD elementwise is ~2x slower per element than DVE;
            # split each mul so both finish together.
            CUT = 512

            def emit_out(p):
                """Output: out = t ⊙ x per half, columns [0:CUT] on DVE,
                [CUT:] on GPSIMD."""
                st[p].pop("svec")
                tp = st[p].pop("tp")
                xp = pre_x[p]
                out_sb = out_pool.tile([P, 2, D], f32, name="out_sb")
                for h in range(2):
                    nc.vector.tensor_mul(
                        out_sb[:, h, 0:CUT], tp[:, h, 0:CUT], xp[:, h, 0:CUT]
                    )
                    nc.gpsimd.tensor_mul(
                        out_sb[:, h, CUT:D], tp[:, h, CUT:D], xp[:, h, CUT:D]
                    )
                if not b_zero:
                    out2 = out_pool.tile([P, 2, D], f32, name="out2")
                    for h in range(2):
                        nc.vector.tensor_add(
                            out2[:, h, :], out_sb[:, h, :], b3bc[:]
                        )
                    out_sb = out2
                st[p]["out"] = out_sb

            def emit_outdma(p):
                out_sb = st.pop(p)["out"]
                r0 = p * 2 * P
                nc.sync.dma_start(out_d[r0:r0 + 2 * P, :], out_sb[:])

            # Software-pipelined emission. Stage lags are chosen so every
            # instruction's producers finished >= 1 iteration earlier.
            # Dep-old stages (out) are emitted BEFORE the iteration's
            # transposes/copies so each engine's in-order stream fills its
            # wait-for-PE window with ready work instead of idling in it.
            def _stage(f, p):
                if 0 <= p < N_PAIRS:
                    f(p)

            for p in range(N_PAIRS + DMA_LAG + 1):
                _stage(emit_t, p - T_LAG)
                _stage(emit_rec, p - REC_LAG)
                _stage(emit_out, p - OUT_LAG)
                _stage(emit_transposes, p)
                _stage(emit_dots, p - DOTS_LAG)
                _stage(emit_outdma, p - DMA_LAG)
    nc.compile()
    return nc


def _get_nc(b_zero: bool):
    if b_zero not in _built:
        _built[b_zero] = _build_nc(b_zero)
    return _built[b_zero]


def _host_prep(w, b, b_zero):
    # Wpk[p, c*4+i] packs column i of [ones, w0, w1, w2] for D-chunk c
    M = np.empty((D, 4), dtype=np.float32)
    M[:, 0] = 1.0
    M[:, 1] = w[0]
    M[:, 2] = w[1]
    M[:, 3] = w[2]
    wpk = np.ascontiguousarray(
        M.reshape(N_CHUNKS, P, 4).transpose(1, 0, 2).reshape(P, N_CHUNKS * 4)
    )
    w3bc = np.ascontiguousarray(np.broadcast_to(w[3], (P, D)).astype(np.float32))
    ident = np.eye(P, dtype=np.float32)
    extras = {}
    if not b_zero:
        c = b.sum(axis=1).astype(np.float32)  # (L,)
        extras["cvec"] = np.ascontiguousarray(np.broadcast_to(c, (P, L)))
        extras["b3bc"] = np.ascontiguousarray(
            np.broadcast_to(b[3], (P, D)).astype(np.float32)
        )
    return wpk, w3bc, ident, extras


def kernel(inputs, w, b):
    from concourse.bass_utils import run_bass_kernel_spmd

    x = np.ascontiguousarray(np.asarray(inputs, dtype=np.float32).reshape(B, D))
    w = np.asarray(w, dtype=np.float32)
    b = np.asarray(b, dtype=np.float32)
    b_zero = not b.any()

    nc = _get_nc(b_zero)
    wpk, w3bc, ident, extras = _host_prep(w, b, b_zero)

    in_maps = []
    for i in range(N_CORES):
        m = {
            "x": x[i * RPC:(i + 1) * RPC],
            "wpk": wpk,
            "w3bc": w3bc,
            "ident": ident,
        }
        m.update(extras)
        in_maps.append(m)

    trace = bool(int(os.environ.get("KERNEL_TRACE", "0")))
    kwargs = {}
    if trace:
        kwargs = {"trace": True, "trace_cores": [0]}
    res = run_bass_kernel_spmd(nc, in_maps, core_ids=list(range(N_CORES)), **kwargs)
    if trace:
        kernel.last_results = res
    return np.concatenate([r["out"] for r in res.results], axis=0)


# revision 18
# speedup vs baseline: 1.1588x; 1.1484x over previous
"""Trainium2 Bass kernel for the CrossLayer problem.

Math: reference computes, per row x (length D), with cur_0 = x:
    cur_{i+1} = sum(cur_i) * (w_i ⊙ x) + b_i + x        (i = 0..L-1)
Only the scalar s_i = sum(cur_i) couples elements, so with
    X   = sum(x)                  (per row)
    W_i = x · w_i                 (per row, i = 0..L-2)
    c_i = sum(b_i)
the recursion collapses to scalars:
    S_0 = X;  S_{i+1} = S_i * W_i + c_i + X
and the output is a single elementwise pass:
    out = x ⊙ (S_{L-1} * w_{L-1} + 1) + b_{L-1}
        = (x ⊙ w_{L-1}) * S_{L-1} + x + b_{L-1}

Kernel layout (per core, pure data parallel over batch, 8 pairs of
(128, 1024) row tiles):
  - The whole 8 MiB input and 8 MiB output stay resident in SBUF (no
    buffer reuse), so every load doorbell issues at t=0 with no semaphore
    wait and the SDMA engines drain the full input back-to-back; store
    doorbells never wait on buffer frees, only on their producer.
  - PE transposes each 128x128 chunk of both tiles of a pair into 4 PSUM
    banks; PSUM->SBUF xT copies ([128,1024] each, fp32 -> f32r cast) go
    one on ACT and one on DVE. Dots [X, W0, W1, W2] via 8 accumulating
    f32r matmuls with N=256 moving; small PE transposes put them
    row-major.
  - The scalar recursion S3 runs as ONE tensor_tensor_scan per half on
    DVE. Both PSUM->SBUF xT copies ride ACT (PSUM reads are 1x on every
    engine). Both t = S3*w3 + 1 halves run on DVE as tensor_scalar,
    which hits the fp32 2x_2P perf mode (SBUF-only, single tensor input,
    both read ports) - 0.53 us/half instead of 1.0+. The muls t ⊙ x are
    1x tensor_tensor ops, split ~50/50 by columns between DVE and
    GPSIMD (GPSIMD is ~2x slower per element).
  - ALL DMA doorbells ride the sync (SP) queue. Interleaved row pairing
    (partition r of tile h holds DRAM row base + 2r + h) lets one
    dma_start move a whole 1MB pair each way.
"""

import os
import numpy as np

B, D, L = 16384, 1024, 4
N_CORES = 8
RPC = B // N_CORES          # rows per core
P = 128                     # partitions
N_TILES = RPC // P          # 16
N_PAIRS = N_TILES // 2      # 8
N_CHUNKS = D // P           # 8

_built = {}


def _build_nc(b_zero: bool):
    import concourse.bass as bass
    import concourse.bacc as bacc
    import concourse.mybir as mybir
    from concourse import tile

    f32 = mybir.dt.float32
    f32r = mybir.dt.float32r
    Alu = mybir.AluOpType
    Act = mybir.ActivationFunctionType

    # Bacc (not raw Bass): its compile() legalizes semaphore waits — TRN2
    # matmuls encode at most one sync wait (walrus S3_LW struct).
    nc = bacc.Bacc(
        "TRN2", target_bir_lowering=False, debug=False, num_devices=N_CORES
    )
    x_d = nc.dram_tensor("x", [RPC, D], f32, kind="ExternalInput")
    wpk_d = nc.dram_tensor("wpk", [P, N_CHUNKS * 4], f32r, kind="ExternalInput")
    w3bc_d = nc.dram_tensor("w3bc", [P, D], f32, kind="ExternalInput")
    ident_d = nc.dram_tensor("ident", [P, P], f32, kind="ExternalInput")
    if not b_zero:
        cvec_d = nc.dram_tensor("cvec", [P, 4], f32, kind="ExternalInput")
        b3bc_d = nc.dram_tensor("b3bc", [P, D], f32, kind="ExternalInput")
    out_d = nc.dram_tensor("out", [RPC, D], f32, kind="ExternalOutput")

    # Stage lags (iterations behind the pair's transposes).
    DOTS_LAG, REC_LAG, T_LAG, OUT_LAG, DMA_LAG = 1, 2, 3, 4, 5

    with tile.TileContext(nc) as tc:
        with (
            tc.tile_pool(name="consts", bufs=1) as consts,
            tc.tile_pool(name="xin", bufs=N_PAIRS) as xin_pool,
            tc.tile_pool(name="tp", bufs=3) as t_pool,
            tc.tile_pool(name="xts", bufs=2) as xts_pool,
            tc.tile_pool(name="outp", bufs=N_PAIRS if b_zero else 4) as out_pool,
            tc.tile_pool(name="small", bufs=5) as small_pool,
            tc.tile_pool(name="ps_t", bufs=2, space=bass.MemorySpace.PSUM) as ps_t,
            tc.tile_pool(name="ps_d", bufs=3, space=bass.MemorySpace.PSUM) as ps_d,
            tc.tile_pool(name="ps_s", bufs=1, space=bass.MemorySpace.PSUM) as ps_s,
        ):
            pre_x = {}

            def load_pair(p, split):
                # Interleaved row pairing: partition r of tile h holds DRAM
                # row p*256 + 2r + h, so one dma_start covers the whole pair
                # (the [P, 2, D] SBUF pattern matches DRAM row-major 1:1).
                xp = xin_pool.tile([P, 2, D], f32, name="xp")
                r0 = p * 2 * P
                if split:
                    for q in range(2):
                        nc.sync.dma_start(
                            xp[:, :, q * (D // 2):(q + 1) * (D // 2)],
                            x_d[r0:r0 + 2 * P, q * (D // 2):(q + 1) * (D // 2)],
                        )
                else:
                    nc.sync.dma_start(xp[:], x_d[r0:r0 + 2 * P, :])
                pre_x[p] = xp

            # consts first (ident gates the very first transpose)
            ident = consts.tile([P, P], f32)
            nc.sync.dma_start(ident[:], ident_d[:])
            wpk = consts.tile([P, N_CHUNKS * 4], f32r)
            nc.sync.dma_start(wpk[:], wpk_d[:])

            load_pair(0, split=True)

            w3bc = consts.tile([P, D], f32)
            nc.sync.dma_start(w3bc[:], w3bc_d[:])
            if not b_zero:
                cvec = consts.tile([P, 4], f32)
                nc.sync.dma_start(cvec[:], cvec_d[:])
                b3bc = consts.tile([P, D], f32)
                nc.sync.dma_start(b3bc[:], b3bc_d[:])

            # ALL remaining pair loads issue back-to-back right here (~0.65 us
            # per doorbell on SP, vs ~2.7 us SDMA drain per 1 MiB pair, so
            # issue stays ahead of the engines). None has a semaphore wait.
            for p in range(1, N_PAIRS):
                load_pair(p, split=False)

            # Prologue: absorb each const-DMA completion into one engine
            # observation up front, so steady-state instructions never need
            # two fresh semaphore waits (walrus: one sync wait per matmul).
            prol0 = ps_t.tile([P, 1024], f32, name="prol0", tag="xt_ps")
            nc.tensor.transpose(prol0[0:P, 0:P], ident[:], ident[:])
            prol1 = ps_d.tile([4, 2 * P], f32, name="prol1", tag="dots_ps")
            nc.tensor.matmul(
                prol1[:, 0:32], wpk[:, 0:4], wpk[:], start=True, stop=True
            )
            prolc = small_pool.tile([P, 1], f32, name="prolc")
            nc.scalar.activation(prolc[:], w3bc[:, 0:1], Act.Copy)
            prolv = small_pool.tile([P, 1], f32, name="prolv")
            nc.vector.tensor_mul(prolv[:], w3bc[:, 0:1], w3bc[:, 0:1])
            prolg = small_pool.tile([P, 1], f32, name="prolg")
            nc.gpsimd.tensor_copy(prolg[:], w3bc[:, 0:1])
            if not b_zero:
                prolg2 = small_pool.tile([P, 1], f32, name="prolg2")
                nc.gpsimd.tensor_copy(prolg2[:], cvec[:, 0:1])
                prolb = small_pool.tile([P, 1], f32, name="prolb")
                nc.vector.tensor_mul(prolb[:], b3bc[:, 0:1], b3bc[:, 0:1])

            # Per-pair state carried between pipeline stages
            st = {}

            def emit_transposes(p):
                """PE: 16 chunk transposes into 2 two-bank PSUM tiles; both
                [128,1024] PSUM->SBUF copies (f32r cast) on ACT."""
                xp = pre_x[p]
                st[p] = {}
                xts = xts_pool.tile([P, 4 * 512], f32r, name="xts")
                for k in range(2):
                    xt_ps = ps_t.tile([P, 1024], f32, name="xt_ps", tag="xt_ps")
                    for cc in range(4):
                        c = 4 * k + cc
                        for h in range(2):
                            nc.tensor.transpose(
                                xt_ps[:, cc * 256 + h * P:cc * 256 + (h + 1) * P],
                                xp[:, h, c * P:(c + 1) * P],
                                ident[:],
                            )
                    nc.scalar.copy(xts[:, k * 1024:(k + 1) * 1024], xt_ps[:])
                st[p]["xts"] = xts

            def emit_dots(p):
                """PE: 8 accumulating f32r matmuls (N=256) + 2 small
                transposes; ACT does the small PSUM->SBUF copies so DVE's
                big k1 copy never delays the PE's dT transpose."""
                xts = st[p]["xts"]
                dots_ps = ps_d.tile([4, 2 * P], f32, name="dots_ps", tag="dots_ps")
                for c in range(N_CHUNKS):
                    nc.tensor.matmul(
                        dots_ps[:],
                        wpk[:, c * 4:(c + 1) * 4],
                        xts[:, c * 256:(c + 1) * 256],
                        start=(c == 0),
                        stop=(c == N_CHUNKS - 1),
                    )
                dots = small_pool.tile([4, 2 * P], f32, name="dots")
                nc.scalar.copy(dots[:], dots_ps[:])
                dT_ps = ps_s.tile([P, 8], f32, name="dT_ps")
                for h in range(2):
                    nc.tensor.transpose(
                        dT_ps[:, h * 4:(h + 1) * 4],
                        dots[:, h * P:(h + 1) * P],
                        ident[0:4, 0:4],
                    )
                dT = small_pool.tile([P, 8], f32, name="dT")
                nc.scalar.copy(dT[:], dT_ps[:])
                st[p]["dT"] = dT
                del st[p]["xts"]

            def emit_rec(p):
                """DVE: the whole scalar recursion as ONE tensor_tensor_scan
                per half (a DVE-only opcode; tiny [P,3] ops)."""
                dT = st[p].pop("dT")
                svec = small_pool.tile([P, 8], f32, name="svec")
                for h in range(2):
                    X = dT[:, 4 * h:4 * h + 1]
                    if b_zero:
                        data1 = X.broadcast_to([P, 3])
                    else:
                        avec = small_pool.tile([P, 8], f32, name="avec")
                        nc.vector.tensor_add(
                            avec[:, 4 * h:4 * h + 3],
                            X.broadcast_to([P, 3]),
                            cvec[:, 0:3],
                        )
                        data1 = avec[:, 4 * h:4 * h + 3]
                    nc.vector.tensor_tensor_scan(
                        svec[:, 4 * h:4 * h + 3],
                        dT[:, 4 * h + 1:4 * h + 4],
                        data1,
                        X,
                        Alu.mult,
                        Alu.add,
                    )
                st[p]["svec"] = svec

            def emit_t(p):
                """DVE: t_h = S3_h*w3 + 1 per half via tensor_scalar —
                single-tensor-input fp32 op from SBUF hits the 2x_2P DVE
                perf mode (~0.53 us/half)."""
                svec = st[p]["svec"]
                tp = t_pool.tile([P, 2, D], f32, name="tp")
                for h in range(2):
                    nc.vector.tensor_scalar(
                        tp[:, h, :], w3bc[:], svec[:, 4 * h + 2:4 * h + 3],
                        1.0, Alu.mult, Alu.add,
                    )
                st[p]["tp"] = tp

            def emit_out(p):
                """DVE: out = t ⊙ x as ONE [P,2048] tensor_tensor op.
                Concurrent DVE+GPSIMD elementwise on the same tiles
                throttles BOTH engines (shared SBUF ports) below what DVE
                achieves alone, so the whole mul stays on DVE."""
                st[p].pop("svec")
                tp = st[p].pop("tp")
                xp = pre_x[p]
                out_sb = out_pool.tile([P, 2, D], f32, name="out_sb")
                nc.vector.tensor_mul(out_sb[:], tp[:], xp[:])
                if not b_zero:
                    out2 = out_pool.tile([P, 2, D], f32, name="out2")
                    for h in range(2):
                        nc.vector.tensor_add(
                            out2[:, h, :], out_sb[:, h, :], b3bc[:]
                        )
                    out_sb = out2
                st[p]["out"] = out_sb

            def emit_outdma(p):
                out_sb = st.pop(p)["out"]
                r0 = p * 2 * P
                nc.sync.dma_start(out_d[r0:r0 + 2 * P, :], out_sb[:])

            # Software-pipelined emission. Stage lags are chosen so every
            # instruction's producers finished >= 1 iteration earlier.
            # Dep-old stages (out) are emitted BEFORE the iteration's
            # transposes/copies so each engine's in-order stream fills its
            # wait-for-PE window with ready work instead of idling in it.
            def _stage(f, p):
                if 0 <= p < N_PAIRS:
                    f(p)

            for p in range(N_PAIRS + DMA_LAG + 1):
                _stage(emit_t, p - T_LAG)
                _stage(emit_rec, p - REC_LAG)
                _stage(emit_out, p - OUT_LAG)
                _stage(emit_transposes, p)
                _stage(emit_dots, p - DOTS_LAG)
                _stage(emit_outdma, p - DMA_LAG)
    nc.compile()
    return nc


def _get_nc(b_zero: bool):
    if b_zero not in _built:
        _built[b_zero] = _build_nc(b_zero)
    return _built[b_zero]


def _host_prep(w, b, b_zero):
    # Wpk[p, c*4+i] packs column i of [ones, w0, w1, w2] for D-chunk c
    M = np.empty((D, 4), dtype=np.float32)
    M[:, 0] = 1.0
    M[:, 1] = w[0]
    M[:, 2] = w[1]
    M[:, 3] = w[2]
    wpk = np.ascontiguousarray(
        M.reshape(N_CHUNKS, P, 4).transpose(1, 0, 2).reshape(P, N_CHUNKS * 4)
    )
    w3bc = np.ascontiguousarray(np.broadcast_to(w[3], (P, D)).astype(np.float32))
    ident = np.eye(P, dtype=np.float32)
    extras = {}
    if not b_zero:
        c = b.sum(axis=1).astype(np.float32)  # (L,)
        extras["cvec"] = np.ascontiguousarray(np.broadcast_to(c, (P, L)))
        extras["b3bc"] = np.ascontiguousarray(
            np.broadcast_to(b[3], (P, D)).astype(np.float32)
        )
    return wpk, w3bc, ident, extras


def kernel(inputs, w, b):
    from concourse.bass_utils import run_bass_kernel_spmd

    x = np.ascontiguousarray(np.asarray(inputs, dtype=np.float32).reshape(B, D))
    w = np.asarray(w, dtype=np.float32)
    b = np.asarray(b, dtype=np.float32)
    b_zero = not b.any()

    nc = _get_nc(b_zero)
    wpk, w3bc, ident, extras = _host_prep(w, b, b_zero)

    in_maps = []
    for i in range(N_CORES):
        m = {
            "x": x[i * RPC:(i + 1) * RPC],
            "wpk": wpk,
            "w3bc": w3bc,
            "ident": ident,
        }
        m.update(extras)
        in_maps.append(m)

    trace = bool(int(os.environ.get("KERNEL_TRACE", "0")))
    kwargs = {}
    if trace:
        kwargs = {"trace": True, "trace_cores": [0]}
    res = run_bass_kernel_spmd(nc, in_maps, core_ids=list(range(N_CORES)), **kwargs)
    if trace:
        kernel.last_results = res
    return np.concatenate([r["out"] for r in res.results], axis=0)
